# revision 20
# baseline (speedup 1.0000x reference)
"""Trainium2 Bass kernel for nn_Attention_18631568130798.

Mixed template/search attention (Stark-style tracker attention):
  qkv proj -> per-head scores + RPE bias -> template-block softmax ->
  boxmask-weighted factor -> rescaled template->search scores -> softmax ->
  attn @ v -> output proj.

Sharding: data-parallel over batch B=64 across 8 NeuronCores (8 batches/core).

v2 design (vs f32r baseline):
  * all-bf16 dataflow (PE matmuls run 1 cycle/row at ANY free size; DVE ops
    at 2x 16-bit rate; half the SBUF/DMA traffic). Host numpy estimate of
    the full-bf16 pipeline error: 4.4e-3 relmax (gate 2e-2).
  * RPE bias gather done on HOST (pure function of rpe_table/rpe_index
    inputs) and DMA'd as a ready bf16 [j, h, i] table - kills the gpsimd
    gather + DRAM bounce preamble.
  * ones-row appended to v and bm lhsT tiles: attention @ v matmuls emit
    softmax denominators as an extra output row - no separate denominator
    matmuls.
  * per-batch-PAIR processing with double-buffered tile pools so consecutive
    pairs/batches overlap across engines.
  * scores are computed TRANSPOSED: ST[j, i] = k_j . q_i, making the
    out^T = (c-part, i-free) of attn@v exactly the lhsT the output
    projection needs. No PE transposes anywhere.
"""

import os
import numpy as np
import ml_dtypes

import concourse.bass as bass
import concourse.mybir as mybir
import concourse.tile as tile
from concourse import bacc
from concourse.bass_utils import run_bass_kernel_spmd

AF = mybir.ActivationFunctionType
F32 = mybir.dt.float32
BF16 = mybir.dt.bfloat16
BF = ml_dtypes.bfloat16

# Problem constants (hardcoded per contract)
B, N, C = 64, 245, 768
H, HD = 12, 64
NT, NS = 49, 196
SCALE = HD ** -0.5
NCORES = 8
NB = B // NCORES              # batches per core
JCH = [(0, 128), (128, 117)]  # j (key-token) partition chunks
# 3-head psum groups with constant partition-half parity s=h%2
HGROUPS = [(0, [0, 2, 4]), (3, [6, 8, 10]), (0, [1, 3, 5]), (3, [7, 9, 11])]


def _build(nb: int, reps: int = 1):
    nc = bacc.Bacc(None, target_bir_lowering=False, name="attn18631v2")

    xT_d = nc.dram_tensor("xT", [nb, C, N], BF16, kind="ExternalInput")
    bm_d = nc.dram_tensor("bm", [nb, NT, H, HD], BF16, kind="ExternalInput")
    wqkT_d = nc.dram_tensor("wqkT", [C, 2 * C], BF16, kind="ExternalInput")
    wvT_d = nc.dram_tensor("wvT", [C, C], BF16, kind="ExternalInput")
    pwT_d = nc.dram_tensor("pwT", [C, C], BF16, kind="ExternalInput")
    bwT_d = nc.dram_tensor("bwT", [C, H], BF16, kind="ExternalInput")
    projb_d = nc.dram_tensor("projb", [1, C], BF16, kind="ExternalInput")
    boxb_d = nc.dram_tensor("boxb", [1, H], BF16, kind="ExternalInput")
    biasT_d = nc.dram_tensor("biasT", [N, H, N], BF16, kind="ExternalInput")
    out_d = nc.dram_tensor("out", [nb, N, C], F32, kind="ExternalOutput")

    with tile.TileContext(nc) as tc:
        res = tc.alloc_tile_pool(name="res", bufs=1)       # resident singles
        xp = tc.alloc_tile_pool(name="xp", bufs=2)
        qkp = tc.alloc_tile_pool(name="qkp", bufs=2)
        ep = tc.alloc_tile_pool(name="ep", bufs=2)
        ap2 = tc.alloc_tile_pool(name="ap2", bufs=2)
        vp = tc.alloc_tile_pool(name="vp", bufs=2)
        bp = tc.alloc_tile_pool(name="bp", bufs=2)
        sm = tc.alloc_tile_pool(name="sm", bufs=2)         # template smalls
        smr = tc.alloc_tile_pool(name="smr", bufs=3)       # per-head normalize smalls
        ep2 = tc.alloc_tile_pool(name="ep2", bufs=2)       # pre-exp scores staging
        yp = tc.alloc_tile_pool(name="yp", bufs=2)         # output staging
        op = tc.alloc_tile_pool(name="op", bufs=2)
        ps = tc.alloc_tile_pool(name="ps", bufs=3, space="PSUM")
        pt = tc.alloc_tile_pool(name="pt", bufs=3, space="PSUM")
        po = tc.alloc_tile_pool(name="po", bufs=2, space="PSUM")

        # ---------------- preamble: constants + weights ----------------
        ones = res.tile([1, 294], BF16, tag="ones")
        nc.vector.memset(ones, 1.0)
        # row masks for the template->search rescale over j partitions
        maskts = res.tile([1, 128], BF16, tag="maskts")
        nc.vector.memset(maskts, 0.0)
        nc.vector.memset(maskts[0:1, NT:128], 1.0)
        imaskts = res.tile([1, 128], BF16, tag="imaskts")
        nc.vector.memset(imaskts, 1.0)
        nc.vector.memset(imaskts[0:1, NT:128], 0.0)

        wqkT = res.tile([128, 6, 2 * C], BF16, tag="wqkT")
        nc.sync.dma_start(out=wqkT, in_=wqkT_d.rearrange("(cc p) o -> p cc o", p=128))
        wvT = res.tile([128, 6, 2, 384], BF16, tag="wvT")
        nc.sync.dma_start(out=wvT, in_=wvT_d.rearrange("(cc p) (hf o) -> p cc hf o", p=128, hf=2))
        pwT = res.tile([128, 6, 2, 384], BF16, tag="pwT")
        nc.sync.dma_start(out=pwT, in_=pwT_d.rearrange("(cc p) (hf o) -> p cc hf o", p=128, hf=2))
        bwT = res.tile([128, 6, H], BF16, tag="bwT")
        nc.sync.dma_start(out=bwT, in_=bwT_d.rearrange("(cc p) h -> p cc h", p=128))
        projb = res.tile([1, C], BF16, tag="projb")
        nc.sync.dma_start(out=projb, in_=projb_d[:])
        boxb = res.tile([1, H], BF16, tag="boxb")
        nc.sync.dma_start(out=boxb, in_=boxb_d[:])
        # host-gathered RPE bias, transposed layout biasT[ch][j, h, i]
        biasT = []
        for ch, (j0, jw) in enumerate(JCH):
            bt = res.tile([128, H, N], BF16, tag=f"biasT{ch}")
            biasT.append(bt)
            nc.sync.dma_start(out=bt[0:jw], in_=biasT_d[j0:j0 + jw])

        # ---------------- main per-batch-pair loop ----------------
        assert nb % 2 == 0
        npairs = (nb // 2) * reps
        for pair0 in range(npairs):
            pair = pair0 % (nb // 2)
            bpair = [2 * pair, 2 * pair + 1]

            xpair = xp.tile([128, 6, 2, N], BF16, tag="xpair")
            for t, b in enumerate(bpair):
                nc.sync.dma_start(out=xpair[:, :, t, :],
                                  in_=xT_d[b].rearrange("(cc p) i -> p cc i", p=128))

            # qk projection (transposed): qkT[p, m, t, i]
            # m 0..5 = q heads (pre-scaled on host), 6..11 = k heads;
            # head h lives at (m = h//2 (+6 for k), partition half s = h%2).
            qkT = qkp.tile([128, 12, 2, N], BF16, tag="qkT")
            for m in range(12):
                p_qk = ps.tile([128, 2, N], F32, tag="mm")
                for cc in range(6):
                    nc.tensor.matmul(p_qk[:], wqkT[:, cc, 128 * m:128 * m + 128],
                                     xpair[:, cc, :, :],
                                     start=(cc == 0), stop=(cc == 5))
                nc.scalar.copy(qkT[:, m, :, :], p_qk[:])

            # boxmask values with ones column (template path lhsT)
            bm_ext = bp.tile([NT, 2, H, HD + 1], BF16, tag="bm")
            nc.vector.memset(bm_ext[:, :, :, HD:HD + 1], 1.0)
            for t, b in enumerate(bpair):
                nc.sync.dma_start(out=bm_ext[0:NT, t, :, 0:HD], in_=bm_d[b])

            # ---- scores + bias -> E = exp(.); A = pre-exp copy of template cols
            # chunk 0 first; the template chain then overlaps chunk 1 scores
            # and the v projection on PE.
            E = [ep.tile([128, H, 2, N], BF16, tag=f"E{ch}", name=f"E{ch}") for ch in range(2)]
            A = [ap2.tile([128, H, 2, NT], BF16, tag=f"A{ch}", name=f"A{ch}") for ch in range(2)]

            def scores_group(ch, g):
                j0, jw = JCH[ch]
                tq0, heads = HGROUPS[g]
                hsl = slice(heads[0], min(heads[0] + 6, H), 2)
                epre = ep2.tile([128, 3, 2, N], BF16, tag="epre")
                for u, h in enumerate(heads):
                    s, mq, mk = h % 2, h // 2, 6 + h // 2
                    p_st = ps.tile([128, 2, N], F32, tag="mm")
                    for t in range(2):
                        nc.tensor.matmul(p_st[0:jw, t, :],
                                         qkT[64 * s:64 * s + 64, mk, t, j0:j0 + jw],
                                         qkT[64 * s:64 * s + 64, mq, t, :],
                                         start=True, stop=True)
                    bb = biasT[ch][0:jw, h, :].rearrange("p (u i) -> p u i", u=1) \
                        .to_broadcast((jw, 2, N))
                    nc.vector.tensor_add(epre[0:jw, u, :, :], p_st[0:jw], bb)
                nc.gpsimd.tensor_copy(A[ch][0:jw, hsl, :, :], epre[0:jw, :, :, 0:NT])
                nc.scalar.activation(E[ch][0:jw, hsl, :, :], epre[0:jw], AF.Exp)

            # ---- template path: out_t^T, denominators via ones row
            rTt = sm.tile([1, H, 2, NT], BF16, tag="rTt")
            otn = sm.tile([128, 6, 2, NT], BF16, tag="otn")

            def template_group_a(g):
                """p_ot matmuls + denominator reciprocal; returns the psum tile."""
                tq0, heads = HGROUPS[g]
                p_ot = pt.tile([65, 3, 2, NT], F32, tag="tiny")
                for u, h in enumerate(heads):
                    for t in range(2):
                        nc.tensor.matmul(p_ot[:, u, t, :],
                                         bm_ext[0:NT, t, h, :],
                                         E[0][0:NT, h, t, 0:NT],
                                         start=True, stop=True)
                hsl = slice(heads[0], min(heads[0] + 6, H), 2)
                dtmp = sm.tile([1, 3, 2, NT], F32, tag="dtmp")
                nc.vector.tensor_scalar_add(dtmp, p_ot[64:65, :, :, :], float(N - NT))
                with nc.allow_low_precision("bf16 reciprocal feeds bf16 matmul"):
                    nc.vector.reciprocal(rTt[0:1, hsl, :, :], dtmp)
                return p_ot

            def template_group_b(g, p_ot):
                """broadcast the reciprocal + normalize (staggered after a).
                p_rbt borrows the attnv psum ring (free during the template
                phase) so the "tiny" ring never self-deadlocks."""
                tq0, heads = HGROUPS[g]
                sgrp = heads[0] % 2
                hsl = slice(heads[0], min(heads[0] + 6, H), 2)
                p_rbt = po.tile([64, 3, 2, NT], F32, tag="ot")
                nc.tensor.matmul(p_rbt[:], ones[0:1, 0:64], rTt[0:1, hsl, :, :],
                                 start=True, stop=True)
                rb_t = sm.tile([64, 3, 2, NT], BF16, tag="rbt")
                nc.scalar.copy(rb_t, p_rbt)
                nc.vector.tensor_mul(otn[64 * sgrp:64 * sgrp + 64, tq0:tq0 + 3, :, :],
                                     p_ot[0:64, :, :, :], rb_t)

            # interleave: dependent template stages hide behind independent
            # scores matmuls in each engine queue
            scores_group(0, 0)
            scores_group(0, 1)
            pot0 = template_group_a(0)
            scores_group(0, 2)
            pot1 = template_group_a(1)
            template_group_b(0, pot0)
            scores_group(0, 3)
            pot2 = template_group_a(2)
            template_group_b(1, pot1)
            pot3 = template_group_a(3)
            template_group_b(2, pot2)
            template_group_b(3, pot3)

            # factor[h, t, i] = box head projection of normalized template out
            p_f = pt.tile([H, 2, NT], F32, tag="tiny")
            for cc in range(6):
                nc.tensor.matmul(p_f[:], bwT[:, cc, :], otn[:, cc, :, :],
                                 start=(cc == 0), stop=False)
            nc.tensor.matmul(p_f[:], boxb[0:1, :], ones[0:1, 0:2 * NT],
                             start=False, stop=True)
            facT = sm.tile([H, 2, NT], BF16, tag="facT")
            nc.scalar.copy(facT, p_f)
            facflat = sm.tile([1, H, 2, NT], BF16, tag="facflat")
            nc.scalar.dma_start(out=facflat[:], in_=facT[:])

            # chunk-1 scores + v projection overlap the template chain on PE
            for g in range(4):
                scores_group(1, g)

            # v projection (row-major, i on partitions) with ones column 64
            v_sb = vp.tile([128, 2, 2, H, HD + 1], BF16, tag="v")
            nc.vector.memset(v_sb[:, :, :, :, HD:HD + 1], 1.0)
            for t in range(2):
                for ic, (i0, iw) in enumerate(JCH):
                    for hf in range(2):
                        p_v = ps.tile([128, 384], F32, tag="mm")
                        for cc in range(6):
                            nc.tensor.matmul(p_v[0:iw], xpair[:, cc, t, i0:i0 + iw],
                                             wvT[:, cc, hf, :],
                                             start=(cc == 0), stop=(cc == 5))
                        nc.scalar.copy(
                            v_sb[0:iw, ic, t, 6 * hf:6 * hf + 6, 0:HD],
                            p_v[0:iw].rearrange("p (r d) -> p r d", r=6))

            # rescale template-query cols of E: E[:, h, :, 0:NT] = exp(A * fb)
            # (chunk0 rows j<NT multiply by 1.0 -> recompute of template block)
            for g, (tq0, heads) in enumerate(HGROUPS):
                hsl = slice(heads[0], min(heads[0] + 6, H), 2)
                fsl = facflat[0:1, hsl, :, :]
                fb0 = pt.tile([128, 3, 2, NT], F32, tag="tiny")
                nc.tensor.matmul(fb0[:], maskts[0:1, :], fsl, start=True, stop=False)
                nc.tensor.matmul(fb0[:], imaskts[0:1, :], ones[0:1, 0:6 * NT],
                                 start=False, stop=True)
                nc.vector.tensor_mul(A[0][:, hsl, :, :], A[0][:, hsl, :, :], fb0[:])
                nc.scalar.activation(E[0][:, hsl, :, 0:NT], A[0][:, hsl, :, :], AF.Exp)
                fb1 = pt.tile([128, 3, 2, NT], F32, tag="tiny")
                nc.tensor.matmul(fb1[:], ones[0:1, 0:128], fsl, start=True, stop=True)
                nc.vector.tensor_mul(A[1][0:117, hsl, :, :], A[1][0:117, hsl, :, :],
                                     fb1[0:117])
                nc.scalar.activation(E[1][0:117, hsl, :, 0:NT], A[1][0:117, hsl, :, :],
                                     AF.Exp)

            # ---- attn @ v (denominator = ones row 64) + normalize
            # head order follows rescale group order; normalize runs on
            # DVE/gpsimd only, so the PE stream is pure attn@v matmuls.
            OTn = op.tile([128, 6, 2, N], BF16, tag="OTn")

            def attn_norm(h, p_o, rec):
                s, g = h % 2, h // 2
                p_rb2 = ps.tile([64, 2, N], F32, tag="mm")
                nc.tensor.matmul(p_rb2[:], ones[0:1, 0:64], rec[0:1, :, :],
                                 start=True, stop=True)
                rb2 = smr.tile([64, 2, N], BF16, tag="rb2")
                nc.scalar.copy(rb2, p_rb2)
                nc.vector.tensor_mul(OTn[64 * s:64 * s + 64, g, :, :],
                                     p_o[0:64, :, :], rb2)

            pending = None
            for _, heads in HGROUPS:
                for h in heads:
                    p_o = po.tile([65, 2, N], F32, tag="ot")
                    for t in range(2):
                        for ch, (j0, jw) in enumerate(JCH):
                            nc.tensor.matmul(p_o[:, t, :],
                                             v_sb[0:jw, ch, t, h, :],
                                             E[ch][0:jw, h, t, :],
                                             start=(ch == 0), stop=(ch == 1))
                    rec = smr.tile([1, 2, N], BF16, tag="rec")
                    with nc.allow_low_precision("bf16 reciprocal feeds matmul"):
                        nc.vector.reciprocal(rec, p_o[64:65, :, :])
                    if pending is not None:
                        attn_norm(*pending)
                    pending = (h, p_o, rec)
            attn_norm(*pending)

            # ---- output projection, psum -> sbuf -> DMA
            for t, b in enumerate(bpair):
                for ic, (i0, iw) in enumerate(JCH):
                    y_sb = yp.tile([128, 2, 384], F32, tag="y")
                    for hf in range(2):
                        p_y = ps.tile([128, 384], F32, tag="mm")
                        for cc in range(6):
                            nc.tensor.matmul(p_y[0:iw], OTn[:, cc, t, i0:i0 + iw],
                                             pwT[:, cc, hf, :],
                                             start=(cc == 0), stop=False)
                        nc.tensor.matmul(p_y[0:iw], ones[0:1, 0:iw],
                                         projb[0:1, 384 * hf:384 * hf + 384],
                                         start=False, stop=True)
                        nc.scalar.copy(y_sb[0:iw, hf, :], p_y[0:iw])
                    nc.scalar.dma_start(
                        out=out_d[b, i0:i0 + iw, :],
                        in_=y_sb[0:iw].rearrange("p hf o -> p (hf o)"))

        for p in (po, pt, ps, op, yp, ep2, smr, sm, bp, vp, ap2, ep, qkp, xp, res):
            p.release()

    nc.finalize()
    return nc


_CACHE = {}


def _get_nc(nb, reps=1):
    key = (nb, reps)
    if key not in _CACHE:
        _CACHE[key] = _build(nb, reps)
    return _CACHE[key]


def _prep_core_inputs(x, boxmask_vec, qkv_w, qkv_b, proj_w, proj_b, box_w, box_b,
                      rpe_table, rpe_index):
    """Host-side prep shared across cores (weights + host RPE gather)."""
    assert np.allclose(qkv_b, 0.0), "kernel assumes qkv_b == 0 (spec fill: zeros)"
    Wq = qkv_w[:C] * np.float32(SCALE)
    Wk = qkv_w[C:2 * C]
    Wv = qkv_w[2 * C:]
    wqkT = np.ascontiguousarray(np.concatenate([Wq, Wk], 0).T).astype(BF)
    wvT = np.ascontiguousarray(Wv.T).astype(BF)
    pwT = np.ascontiguousarray(proj_w.T).astype(BF)
    bwT = np.ascontiguousarray(box_w.T).astype(BF)
    # biasT[j, h, i] = rpe_table[h, rpe_index[i, j]]
    biasT = np.ascontiguousarray(
        np.transpose(rpe_table[:, rpe_index.T], (1, 0, 2))).astype(BF)
    return {
        "wqkT": wqkT, "wvT": wvT, "pwT": pwT, "bwT": bwT,
        "projb": np.ascontiguousarray(proj_b[None, :]).astype(BF),
        "boxb": np.ascontiguousarray(box_b[None, :]).astype(BF),
        "biasT": biasT,
    }


def kernel(x, boxmask_vec, qkv_w, qkv_b, proj_w, proj_b, box_w, box_b,
           rpe_table, rpe_index, lens_t, _nb=NB, _trace=False, _reps=1):
    x = np.asarray(x, np.float32)
    boxmask_vec = np.asarray(boxmask_vec, np.float32)
    qkv_w = np.asarray(qkv_w, np.float32)
    qkv_b = np.asarray(qkv_b, np.float32)
    proj_w = np.asarray(proj_w, np.float32)
    proj_b = np.asarray(proj_b, np.float32)
    box_w = np.asarray(box_w, np.float32)
    box_b = np.asarray(box_b, np.float32)
    rpe_table = np.asarray(rpe_table, np.float32)
    rpe_index = np.asarray(rpe_index, np.int32)
    assert int(lens_t) == NT and x.shape == (B, N, C)

    shared = _prep_core_inputs(x, boxmask_vec, qkv_w, qkv_b, proj_w, proj_b,
                               box_w, box_b, rpe_table, rpe_index)
    nb = _nb
    nc = _get_nc(nb, _reps)
    in_maps = []
    for c in range(NCORES):
        bs = [min(c * nb + i, B - 1) for i in range(nb)]
        m = dict(shared)
        m["xT"] = np.ascontiguousarray(x[bs].transpose(0, 2, 1)).astype(BF)
        m["bm"] = np.ascontiguousarray(
            boxmask_vec[bs].reshape(len(bs), NT, H, HD)).astype(BF)
        in_maps.append(m)
    res = run_bass_kernel_spmd(nc, in_maps, list(range(NCORES)),
                               trace=_trace or bool(os.environ.get("BASS_TRACE")))
    out = np.empty((NCORES * nb, N, C), np.float32)
    for c in range(NCORES):
        out[c * nb:(c + 1) * nb] = res.results[c]["out"]
    if _trace:
        kernel._last = res
    return out[:B] if nb == NB else out


# revision 21
# speedup vs baseline: 1.8947x; 1.8947x over previous
"""Trainium2 Bass kernel for nn_Attention_18631568130798.

Mixed template/search attention (Stark-style tracker attention):
  qkv proj -> per-head scores + RPE bias -> template-block softmax ->
  boxmask-weighted factor -> rescaled template->search scores -> softmax ->
  attn @ v -> output proj.

Sharding: data-parallel over batch B=64 across 8 NeuronCores (8 batches/core).

v2 design (vs f32r baseline):
  * all-bf16 dataflow (PE matmuls run 1 cycle/row at ANY free size; DVE ops
    at 2x 16-bit rate; half the SBUF/DMA traffic). Host numpy estimate of
    the full-bf16 pipeline error: 4.4e-3 relmax (gate 2e-2).
  * RPE bias gather done on HOST (pure function of rpe_table/rpe_index
    inputs) and DMA'd as a ready bf16 [j, h, i] table - kills the gpsimd
    gather + DRAM bounce preamble.
  * ones-row appended to v and bm lhsT tiles: attention @ v matmuls emit
    softmax denominators as an extra output row - no separate denominator
    matmuls.
  * per-batch-PAIR processing with double-buffered tile pools so consecutive
    pairs/batches overlap across engines.
  * scores are computed TRANSPOSED: ST[j, i] = k_j . q_i, making the
    out^T = (c-part, i-free) of attn@v exactly the lhsT the output
    projection needs. No PE transposes anywhere.
"""

import os
import numpy as np
import ml_dtypes

import concourse.bass as bass
import concourse.mybir as mybir
import concourse.tile as tile
from concourse import bacc
from concourse.bass_utils import run_bass_kernel_spmd

AF = mybir.ActivationFunctionType
F32 = mybir.dt.float32
BF16 = mybir.dt.bfloat16
BF = ml_dtypes.bfloat16

# Problem constants (hardcoded per contract)
B, N, C = 64, 245, 768
H, HD = 12, 64
NT, NS = 49, 196
SCALE = HD ** -0.5
NCORES = 8
NB = B // NCORES              # batches per core
JCH = [(0, 128), (128, 117)]  # j (key-token) partition chunks
# 3-head psum groups with constant partition-half parity s=h%2
HGROUPS = [(0, [0, 2, 4]), (3, [6, 8, 10]), (0, [1, 3, 5]), (3, [7, 9, 11])]


def _build(nb: int, reps: int = 1):
    nc = bacc.Bacc(None, target_bir_lowering=False, name="attn18631v2")

    xT_d = nc.dram_tensor("xT", [nb, C, N], BF16, kind="ExternalInput")
    bm_d = nc.dram_tensor("bm", [nb, NT, H, HD], BF16, kind="ExternalInput")
    wqkT_d = nc.dram_tensor("wqkT", [C, 2 * C], BF16, kind="ExternalInput")
    wvT_d = nc.dram_tensor("wvT", [C, C], BF16, kind="ExternalInput")
    pwT_d = nc.dram_tensor("pwT", [C, C], BF16, kind="ExternalInput")
    bwT_d = nc.dram_tensor("bwT", [C, H], BF16, kind="ExternalInput")
    projb_d = nc.dram_tensor("projb", [1, C], BF16, kind="ExternalInput")
    boxb_d = nc.dram_tensor("boxb", [1, H], BF16, kind="ExternalInput")
    biasT_d = nc.dram_tensor("biasT", [N, H, N], BF16, kind="ExternalInput")
    out_d = nc.dram_tensor("out", [nb, N, C], F32, kind="ExternalOutput")

    with tile.TileContext(nc) as tc:
        res = tc.alloc_tile_pool(name="res", bufs=1)       # resident singles
        xp = tc.alloc_tile_pool(name="xp", bufs=2)
        qkp = tc.alloc_tile_pool(name="qkp", bufs=2)
        ep = tc.alloc_tile_pool(name="ep", bufs=2)
        ap2 = tc.alloc_tile_pool(name="ap2", bufs=2)
        vp = tc.alloc_tile_pool(name="vp", bufs=2)
        bp = tc.alloc_tile_pool(name="bp", bufs=2)
        sm = tc.alloc_tile_pool(name="sm", bufs=2)         # template smalls
        smr = tc.alloc_tile_pool(name="smr", bufs=3)       # per-head normalize smalls
        ep2 = tc.alloc_tile_pool(name="ep2", bufs=2)       # pre-exp scores staging
        yp = tc.alloc_tile_pool(name="yp", bufs=2)         # output staging
        op = tc.alloc_tile_pool(name="op", bufs=2)
        ps = tc.alloc_tile_pool(name="ps", bufs=3, space="PSUM")
        pt = tc.alloc_tile_pool(name="pt", bufs=2, space="PSUM")
        po = tc.alloc_tile_pool(name="po", bufs=3, space="PSUM")

        # ---------------- preamble: constants + weights ----------------
        ones = res.tile([1, 294], BF16, tag="ones")
        nc.vector.memset(ones, 1.0)
        # row masks for the template->search rescale over j partitions
        maskts = res.tile([1, 128], BF16, tag="maskts")
        nc.vector.memset(maskts, 0.0)
        nc.vector.memset(maskts[0:1, NT:128], 1.0)
        imaskts = res.tile([1, 128], BF16, tag="imaskts")
        nc.vector.memset(imaskts, 1.0)
        nc.vector.memset(imaskts[0:1, NT:128], 0.0)

        wqkT = res.tile([128, 6, 2 * C], BF16, tag="wqkT")
        nc.sync.dma_start(out=wqkT, in_=wqkT_d.rearrange("(cc p) o -> p cc o", p=128))
        wvT = res.tile([128, 6, 2, 384], BF16, tag="wvT")
        nc.sync.dma_start(out=wvT, in_=wvT_d.rearrange("(cc p) (hf o) -> p cc hf o", p=128, hf=2))
        pwT = res.tile([128, 6, 2, 384], BF16, tag="pwT")
        nc.sync.dma_start(out=pwT, in_=pwT_d.rearrange("(cc p) (hf o) -> p cc hf o", p=128, hf=2))
        bwT = res.tile([128, 6, H], BF16, tag="bwT")
        nc.sync.dma_start(out=bwT, in_=bwT_d.rearrange("(cc p) h -> p cc h", p=128))
        projb = res.tile([1, C], BF16, tag="projb")
        nc.sync.dma_start(out=projb, in_=projb_d[:])
        boxb = res.tile([1, H], BF16, tag="boxb")
        nc.sync.dma_start(out=boxb, in_=boxb_d[:])
        # host-gathered RPE bias, transposed layout biasT[ch][j, h, i]
        biasT = []
        for ch, (j0, jw) in enumerate(JCH):
            bt = res.tile([128, H, N], BF16, tag=f"biasT{ch}")
            biasT.append(bt)
            nc.sync.dma_start(out=bt[0:jw], in_=biasT_d[j0:j0 + jw])

        # ---------------- main per-batch-pair loop ----------------
        assert nb % 2 == 0
        npairs = (nb // 2) * reps
        for pair0 in range(npairs):
            pair = pair0 % (nb // 2)
            bpair = [2 * pair, 2 * pair + 1]

            xpair = xp.tile([128, 6, 2, N], BF16, tag="xpair")
            for t, b in enumerate(bpair):
                nc.sync.dma_start(out=xpair[:, :, t, :],
                                  in_=xT_d[b].rearrange("(cc p) i -> p cc i", p=128))

            # qk projection (transposed): qkT[p, m, t, i]
            # m 0..5 = q heads (pre-scaled on host), 6..11 = k heads;
            # head h lives at (m = h//2 (+6 for k), partition half s = h%2).
            qkT = qkp.tile([128, 12, 2, N], BF16, tag="qkT")
            for m in range(12):
                p_qk = ps.tile([128, 2, N], F32, tag="mm")
                for cc in range(6):
                    nc.tensor.matmul(p_qk[:], wqkT[:, cc, 128 * m:128 * m + 128],
                                     xpair[:, cc, :, :],
                                     start=(cc == 0), stop=(cc == 5))
                nc.scalar.copy(qkT[:, m, :, :], p_qk[:])

            # boxmask values with ones column (template path lhsT)
            bm_ext = bp.tile([NT, 2, H, HD + 1], BF16, tag="bm")
            nc.vector.memset(bm_ext[:, :, :, HD:HD + 1], 1.0)
            for t, b in enumerate(bpair):
                nc.sync.dma_start(out=bm_ext[0:NT, t, :, 0:HD], in_=bm_d[b])

            # ---- scores + bias -> E = exp(.); A = pre-exp copy of template cols
            # chunk 0 first; the template chain then overlaps chunk 1 scores
            # and the v projection on PE.
            E = [ep.tile([128, H, 2, N], BF16, tag=f"E{ch}", name=f"E{ch}") for ch in range(2)]
            A = [ap2.tile([128, H, 2, NT], BF16, tag=f"A{ch}", name=f"A{ch}") for ch in range(2)]

            def scores_group(ch, g):
                j0, jw = JCH[ch]
                tq0, heads = HGROUPS[g]
                hsl = slice(heads[0], min(heads[0] + 6, H), 2)
                epre = ep2.tile([128, 3, 2, N], BF16, tag="epre")
                for u, h in enumerate(heads):
                    s, mq, mk = h % 2, h // 2, 6 + h // 2
                    p_st = ps.tile([128, 2, N], F32, tag="mm")
                    for t in range(2):
                        nc.tensor.matmul(p_st[0:jw, t, :],
                                         qkT[64 * s:64 * s + 64, mk, t, j0:j0 + jw],
                                         qkT[64 * s:64 * s + 64, mq, t, :],
                                         start=True, stop=True)
                    bb = biasT[ch][0:jw, h, :].rearrange("p (u i) -> p u i", u=1) \
                        .to_broadcast((jw, 2, N))
                    nc.vector.tensor_add(epre[0:jw, u, :, :], p_st[0:jw], bb)
                nc.gpsimd.tensor_copy(A[ch][0:jw, hsl, :, :], epre[0:jw, :, :, 0:NT])
                nc.scalar.activation(E[ch][0:jw, hsl, :, :], epre[0:jw], AF.Exp)

            # ---- template path: out_t^T, denominators via ones row
            rTt = sm.tile([1, H, 2, NT], BF16, tag="rTt")
            otn = sm.tile([128, 6, 2, NT], BF16, tag="otn")

            def template_group_a(g):
                """p_ot matmuls + denominator reciprocal; returns the psum tile."""
                tq0, heads = HGROUPS[g]
                p_ot = pt.tile([65, 3, 2, NT], F32, tag="tiny")
                for u, h in enumerate(heads):
                    for t in range(2):
                        nc.tensor.matmul(p_ot[:, u, t, :],
                                         bm_ext[0:NT, t, h, :],
                                         E[0][0:NT, h, t, 0:NT],
                                         start=True, stop=True)
                hsl = slice(heads[0], min(heads[0] + 6, H), 2)
                dtmp = sm.tile([1, 3, 2, NT], F32, tag="dtmp")
                nc.vector.tensor_scalar_add(dtmp, p_ot[64:65, :, :, :], float(N - NT))
                with nc.allow_low_precision("bf16 reciprocal feeds bf16 matmul"):
                    nc.vector.reciprocal(rTt[0:1, hsl, :, :], dtmp)
                return p_ot

            def template_group_b(g, p_ot):
                """broadcast the reciprocal + normalize (staggered after a).
                p_rbt borrows the attnv psum ring (free during the template
                phase) so the "tiny" ring never self-deadlocks."""
                tq0, heads = HGROUPS[g]
                sgrp = heads[0] % 2
                hsl = slice(heads[0], min(heads[0] + 6, H), 2)
                p_rbt = po.tile([64, 3, 2, NT], F32, tag="ot")
                nc.tensor.matmul(p_rbt[:], ones[0:1, 0:64], rTt[0:1, hsl, :, :],
                                 start=True, stop=True)
                rb_t = sm.tile([64, 3, 2, NT], BF16, tag="rbt")
                nc.scalar.copy(rb_t, p_rbt)
                nc.vector.tensor_mul(otn[64 * sgrp:64 * sgrp + 64, tq0:tq0 + 3, :, :],
                                     p_ot[0:64, :, :, :], rb_t)

            # interleave: dependent template stages hide behind independent
            # scores matmuls in each engine queue
            scores_group(0, 0)
            scores_group(0, 1)
            pot0 = template_group_a(0)
            scores_group(0, 2)
            pot1 = template_group_a(1)
            template_group_b(0, pot0)
            scores_group(0, 3)
            pot2 = template_group_a(2)
            template_group_b(1, pot1)
            pot3 = template_group_a(3)
            template_group_b(2, pot2)
            template_group_b(3, pot3)

            # factor[h, t, i] = box head projection of normalized template out
            p_f = pt.tile([H, 2, NT], F32, tag="tiny")
            for cc in range(6):
                nc.tensor.matmul(p_f[:], bwT[:, cc, :], otn[:, cc, :, :],
                                 start=(cc == 0), stop=False)
            nc.tensor.matmul(p_f[:], boxb[0:1, :], ones[0:1, 0:2 * NT],
                             start=False, stop=True)
            facT = sm.tile([H, 2, NT], BF16, tag="facT")
            nc.scalar.copy(facT, p_f)
            facflat = sm.tile([1, H, 2, NT], BF16, tag="facflat")
            nc.scalar.dma_start(out=facflat[:], in_=facT[:])

            # chunk-1 scores + v projection overlap the template chain on PE
            for g in range(4):
                scores_group(1, g)

            # v projection (row-major, i on partitions) with ones column 64
            v_sb = vp.tile([128, 2, 2, H, HD + 1], BF16, tag="v")
            nc.vector.memset(v_sb[:, :, :, :, HD:HD + 1], 1.0)
            for t in range(2):
                for ic, (i0, iw) in enumerate(JCH):
                    for hf in range(2):
                        p_v = ps.tile([128, 384], F32, tag="mm")
                        for cc in range(6):
                            nc.tensor.matmul(p_v[0:iw], xpair[:, cc, t, i0:i0 + iw],
                                             wvT[:, cc, hf, :],
                                             start=(cc == 0), stop=(cc == 5))
                        nc.scalar.copy(
                            v_sb[0:iw, ic, t, 6 * hf:6 * hf + 6, 0:HD],
                            p_v[0:iw].rearrange("p (r d) -> p r d", r=6))

            # rescale template-query cols of E: E[:, h, :, 0:NT] = exp(A * fb)
            # (chunk0 rows j<NT multiply by 1.0 -> recompute of template block)
            for g, (tq0, heads) in enumerate(HGROUPS):
                hsl = slice(heads[0], min(heads[0] + 6, H), 2)
                fsl = facflat[0:1, hsl, :, :]
                fb0 = pt.tile([128, 3, 2, NT], F32, tag="tiny")
                nc.tensor.matmul(fb0[:], maskts[0:1, :], fsl, start=True, stop=False)
                nc.tensor.matmul(fb0[:], imaskts[0:1, :], ones[0:1, 0:6 * NT],
                                 start=False, stop=True)
                nc.vector.tensor_mul(A[0][:, hsl, :, :], A[0][:, hsl, :, :], fb0[:])
                nc.scalar.activation(E[0][:, hsl, :, 0:NT], A[0][:, hsl, :, :], AF.Exp)
                fb1 = pt.tile([128, 3, 2, NT], F32, tag="tiny")
                nc.tensor.matmul(fb1[:], ones[0:1, 0:128], fsl, start=True, stop=True)
                nc.vector.tensor_mul(A[1][0:117, hsl, :, :], A[1][0:117, hsl, :, :],
                                     fb1[0:117])
                nc.scalar.activation(E[1][0:117, hsl, :, 0:NT], A[1][0:117, hsl, :, :],
                                     AF.Exp)

            # ---- attn @ v (denominator = ones row 64) + normalize
            # head order follows rescale group order; normalize runs on
            # DVE/gpsimd only, so the PE stream is pure attn@v matmuls.
            OTn = op.tile([128, 6, 2, N], BF16, tag="OTn")

            def attn_norm(h, p_o, rec):
                s, g = h % 2, h // 2
                p_rb2 = ps.tile([64, 2, N], F32, tag="mm")
                nc.tensor.matmul(p_rb2[:], ones[0:1, 0:64], rec[0:1, :, :],
                                 start=True, stop=True)
                rb2 = smr.tile([64, 2, N], BF16, tag="rb2")
                nc.scalar.copy(rb2, p_rb2)
                nc.vector.tensor_mul(OTn[64 * s:64 * s + 64, g, :, :],
                                     p_o[0:64, :, :], rb2)

            pending = None
            for _, heads in HGROUPS:
                for h in heads:
                    p_o = po.tile([65, 2, N], F32, tag="ot")
                    for t in range(2):
                        for ch, (j0, jw) in enumerate(JCH):
                            nc.tensor.matmul(p_o[:, t, :],
                                             v_sb[0:jw, ch, t, h, :],
                                             E[ch][0:jw, h, t, :],
                                             start=(ch == 0), stop=(ch == 1))
                    rec = smr.tile([1, 2, N], BF16, tag="rec")
                    with nc.allow_low_precision("bf16 reciprocal feeds matmul"):
                        nc.vector.reciprocal(rec, p_o[64:65, :, :])
                    if pending is not None:
                        attn_norm(*pending)
                    pending = (h, p_o, rec)
            attn_norm(*pending)

            # ---- output projection, psum -> sbuf -> DMA
            for t, b in enumerate(bpair):
                for ic, (i0, iw) in enumerate(JCH):
                    y_sb = yp.tile([128, 2, 384], F32, tag="y")
                    for hf in range(2):
                        p_y = ps.tile([128, 384], F32, tag="mm")
                        for cc in range(6):
                            nc.tensor.matmul(p_y[0:iw], OTn[:, cc, t, i0:i0 + iw],
                                             pwT[:, cc, hf, :],
                                             start=(cc == 0), stop=False)
                        nc.tensor.matmul(p_y[0:iw], ones[0:1, 0:iw],
                                         projb[0:1, 384 * hf:384 * hf + 384],
                                         start=False, stop=True)
                        nc.scalar.copy(y_sb[0:iw, hf, :], p_y[0:iw])
                    nc.scalar.dma_start(
                        out=out_d[b, i0:i0 + iw, :],
                        in_=y_sb[0:iw].rearrange("p hf o -> p (hf o)"))

        for p in (po, pt, ps, op, yp, ep2, smr, sm, bp, vp, ap2, ep, qkp, xp, res):
            p.release()

    nc.finalize()
    return nc


_CACHE = {}


def _get_nc(nb, reps=1):
    key = (nb, reps)
    if key not in _CACHE:
        _CACHE[key] = _build(nb, reps)
    return _CACHE[key]


def _prep_core_inputs(x, boxmask_vec, qkv_w, qkv_b, proj_w, proj_b, box_w, box_b,
                      rpe_table, rpe_index):
    """Host-side prep shared across cores (weights + host RPE gather)."""
    assert np.allclose(qkv_b, 0.0), "kernel assumes qkv_b == 0 (spec fill: zeros)"
    Wq = qkv_w[:C] * np.float32(SCALE)
    Wk = qkv_w[C:2 * C]
    Wv = qkv_w[2 * C:]
    wqkT = np.ascontiguousarray(np.concatenate([Wq, Wk], 0).T).astype(BF)
    wvT = np.ascontiguousarray(Wv.T).astype(BF)
    pwT = np.ascontiguousarray(proj_w.T).astype(BF)
    bwT = np.ascontiguousarray(box_w.T).astype(BF)
    # biasT[j, h, i] = rpe_table[h, rpe_index[i, j]]
    biasT = np.ascontiguousarray(
        np.transpose(rpe_table[:, rpe_index.T], (1, 0, 2))).astype(BF)
    return {
        "wqkT": wqkT, "wvT": wvT, "pwT": pwT, "bwT": bwT,
        "projb": np.ascontiguousarray(proj_b[None, :]).astype(BF),
        "boxb": np.ascontiguousarray(box_b[None, :]).astype(BF),
        "biasT": biasT,
    }


def kernel(x, boxmask_vec, qkv_w, qkv_b, proj_w, proj_b, box_w, box_b,
           rpe_table, rpe_index, lens_t, _nb=NB, _trace=False, _reps=1):
    x = np.asarray(x, np.float32)
    boxmask_vec = np.asarray(boxmask_vec, np.float32)
    qkv_w = np.asarray(qkv_w, np.float32)
    qkv_b = np.asarray(qkv_b, np.float32)
    proj_w = np.asarray(proj_w, np.float32)
    proj_b = np.asarray(proj_b, np.float32)
    box_w = np.asarray(box_w, np.float32)
    box_b = np.asarray(box_b, np.float32)
    rpe_table = np.asarray(rpe_table, np.float32)
    rpe_index = np.asarray(rpe_index, np.int32)
    assert int(lens_t) == NT and x.shape == (B, N, C)

    shared = _prep_core_inputs(x, boxmask_vec, qkv_w, qkv_b, proj_w, proj_b,
                               box_w, box_b, rpe_table, rpe_index)
    nb = _nb
    nc = _get_nc(nb, _reps)
    in_maps = []
    for c in range(NCORES):
        bs = [min(c * nb + i, B - 1) for i in range(nb)]
        m = dict(shared)
        m["xT"] = np.ascontiguousarray(x[bs].transpose(0, 2, 1)).astype(BF)
        m["bm"] = np.ascontiguousarray(
            boxmask_vec[bs].reshape(len(bs), NT, H, HD)).astype(BF)
        in_maps.append(m)
    res = run_bass_kernel_spmd(nc, in_maps, list(range(NCORES)),
                               trace=_trace or bool(os.environ.get("BASS_TRACE")))
    out = np.empty((NCORES * nb, N, C), np.float32)
    for c in range(NCORES):
        out[c * nb:(c + 1) * nb] = res.results[c]["out"]
    if _trace:
        kernel._last = res
    return out[:B] if nb == NB else out
